# revision 44
# baseline (speedup 1.0000x reference)
"""Llama attention layer (GQA + RoPE + causal softmax + o_proj) on 8 Trainium2
NeuronCores.

Sharding: tensor-parallel across heads. Core c owns q-heads [4c..4c+3] and
kv-head c (GQA group), computes q/k/v projections for those heads, RoPE,
causal attention, and a partial o_proj against its 512 columns of Wo. The
8 partial [S, H] outputs are summed at the unshard step (the "all-reduce
after o_proj" of the TP recipe).

Dataflow on each core is transpose-free:
  - hsT (host-pretransposed [H, S] bf16) feeds the projections with the
    contraction dim H on partitions.
  - q, k are produced transposed: qT/kT = [d, s]; RoPE is applied in this
    layout (rotate_half becomes a 128x128 signed-permutation matmul).
  - scores are computed transposed, scoresT = [k, q] = kT.T-slice @ qT,
    so softmax'd probsT feed the AV matmul directly as rhs and v ([s, d],
    from a PE transpose of vT) as lhsT, producing ctxT = [d, s].
  - ctxT slices are exactly the lhsT the o_proj matmul needs.
  - softmax skips the running max (scores are O(10) here, exp is safe in
    fp32), the denominator l = sum_k exp comes from a ones-vector matmul,
    and 1/l is broadcast along partitions with a rank-1 PE matmul and
    folded into the ctx eviction.

All matmuls run in bf16 with fp32 PSUM accumulation (fp32 matmuls cost 2
cycles/row on TRN2); RoPE and softmax statistics stay in fp32.
"""

import os
import sys

import numpy as np

for _p in ("/opt/trn_rl_repo", "/root/.axon_site/_ro/trn_rl_repo"):
    if os.path.isdir(_p) and _p not in sys.path:
        sys.path.insert(0, _p)

import ml_dtypes

BF16 = ml_dtypes.bfloat16

# Problem shape (hardcoded per contract).
S = 2048          # sequence length
H = 4096          # hidden size
D = 128           # head dim
NQH = 32          # total q heads
NKVH = 8          # total kv heads
NCORES = 8
QH = NQH // NCORES          # q heads per core (4)
DC = QH * D                 # ctx dims per core (512)
SB = 512                    # s-block (matmul free dim)
NSB = S // SB               # 4 s-blocks
NHCH = H // 128             # 32 contraction chunks for projections
NHO = H // SB               # 8 output column blocks for o_proj
SCALE = float(D) ** -0.5

_cache: dict = {}


def _emit_program(nc, tc, mybir, bass):
    """Emit the per-core Tile program (SPMD: identical on all 8 cores)."""
    from contextlib import ExitStack

    f32 = mybir.dt.float32
    bf16 = mybir.dt.bfloat16
    Exp = mybir.ActivationFunctionType.Exp
    mult = mybir.AluOpType.mult
    add = mybir.AluOpType.add

    # DRAM I/O access patterns
    hs_r = nc.dram_tensor("hsT", [H, S], bf16, kind="ExternalInput").ap() \
        .rearrange("(n p) s -> p n s", p=128)                    # [128, 32, S]
    cosT = nc.dram_tensor("cosT", [D, S], bf16, kind="ExternalInput").ap()
    sinT = nc.dram_tensor("sinT", [D, S], bf16, kind="ExternalInput").ap()
    wq_r = nc.dram_tensor("wqT", [H, DC], bf16, kind="ExternalInput").ap() \
        .rearrange("(n p) m -> p n m", p=128)                    # [128, 32, 512]
    wk_r = nc.dram_tensor("wkT", [H, D], bf16, kind="ExternalInput").ap() \
        .rearrange("(n p) m -> p n m", p=128)                    # [128, 32, 128]
    wv_r = nc.dram_tensor("wvT", [H, D], bf16, kind="ExternalInput").ap() \
        .rearrange("(n p) m -> p n m", p=128)
    wo_r = nc.dram_tensor("woT", [DC, H], bf16, kind="ExternalInput").ap() \
        .rearrange("(n p) m -> p n m", p=128)                    # [128, 4, 4096]
    rperm_d = nc.dram_tensor("rperm", [128, 128], f32, kind="ExternalInput").ap()
    ident_d = nc.dram_tensor("ident", [128, 128], bf16, kind="ExternalInput").ap()
    onesc_d = nc.dram_tensor("ones_col", [128, 1], bf16, kind="ExternalInput").ap()
    onesr_d = nc.dram_tensor("ones_row", [1, 128], f32, kind="ExternalInput").ap()
    masks_r = nc.dram_tensor("masks", [SB // 128, 128, SB], bf16,
                             kind="ExternalInput").ap().rearrange("r p q -> p r q")
    out_d = nc.dram_tensor("out", [S, H], bf16, kind="ExternalOutput").ap()

    ctx = ExitStack()
    with ctx:
        consts = ctx.enter_context(tc.tile_pool(name="consts", bufs=1))
        persist = ctx.enter_context(tc.tile_pool(name="persist", bufs=1))
        wpool = ctx.enter_context(tc.tile_pool(name="wpool", bufs=1))
        hs_pool = ctx.enter_context(tc.tile_pool(name="hs_pool", bufs=3))
        acc_pool = ctx.enter_context(tc.tile_pool(name="acc_pool", bufs=1))
        tmp = ctx.enter_context(tc.tile_pool(name="tmp", bufs=2))
        p_pool = ctx.enter_context(tc.tile_pool(name="p_pool", bufs=5))
        # PSUM budget (8 banks): pj tag holds [128, 2, SB] 2-bank slots x2
        # (proj groups + o_proj + paired score tiles), outT 2, misc 2
        # (rope/transpose/bcast + the l accumulator).
        pj_ps = ctx.enter_context(tc.tile_pool(name="pj_ps", bufs=2, space="PSUM"))
        misc_ps = ctx.enter_context(tc.tile_pool(name="misc_ps", bufs=2, space="PSUM"))
        att_ps = ctx.enter_context(tc.tile_pool(name="att_ps", bufs=2, space="PSUM"))

        # Constants (tiles now; DMAs are emitted lazily after the first
        # projection group so the critical first weight/hs loads go first)
        rperm = consts.tile([128, 128], f32, name="rperm")
        ident = consts.tile([128, 128], bf16, name="ident")
        ones_col = consts.tile([128, 1], bf16, name="ones_col")
        ones_row = consts.tile([1, 128], f32, name="ones_row")
        masks = consts.tile([128, SB // 128, SB], bf16, name="masks")
        cosT_t = consts.tile([D, S], bf16, name="cosT")
        sinT_t = consts.tile([D, S], bf16, name="sinT")

        def load_consts():
            nc.sync.dma_start(rperm[:], rperm_d)
            nc.sync.dma_start(ident[:], ident_d)
            nc.sync.dma_start(ones_col[:], onesc_d)
            nc.sync.dma_start(ones_row[:], onesr_d)
            nc.sync.dma_start(masks[:], masks_r)
            nc.sync.dma_start(cosT_t[:], cosT)
            nc.sync.dma_start(sinT_t[:], sinT)

        # Persistent activations
        qT_all = persist.tile([128, QH * S], bf16, name="qT_all")   # per head [d, s]
        kT_all = persist.tile([128, S], bf16, name="kT_all")        # [d, s]
        v_all = persist.tile([128, S], bf16, name="v_all")          # 16 x [s128, d128]
        ctxT_all = persist.tile([128, QH * S], bf16, name="ctxT_all")

        # Weight tiles, loaded in group-granular DMAs pipelined with the
        # first projection pass. GROUPS lists the H-chunk count per PSUM
        # accumulation group; the first groups are small so the very first
        # matmuls wait on less DMA.
        WGROUPS = [2, 2, 4, 8, 8, 8]         # weight DMA granularity (j=0)
        WOFF = [sum(WGROUPS[:g]) for g in range(len(WGROUPS))]
        wq_t = [wpool.tile([128, gsz, DC], bf16, name=f"wq{g}")
                for g, gsz in enumerate(WGROUPS)]
        wk_t = [wpool.tile([128, gsz, D], bf16, name=f"wk{g}")
                for g, gsz in enumerate(WGROUPS)]
        wv_t = [wpool.tile([128, gsz, D], bf16, name=f"wv{g}")
                for g, gsz in enumerate(WGROUPS)]
        wo_t = [wpool.tile([128, H], bf16, name=f"wo{ch}") for ch in range(QH)]

        def _wslice(tiles, ih):
            for g in range(len(WGROUPS) - 1, -1, -1):
                if ih >= WOFF[g]:
                    return tiles[g], ih - WOFF[g]
            raise AssertionError

        def rope_emit(acc, dst_slice, j):
            """dst(bf16) = acc*cosT + rotate_half(acc)*sinT for s-block j.

            acc is the SBUF f32 projection accumulator; rotate_half is a
            signed 128x128 permutation done on the PE."""
            rh = misc_ps.tile([128, SB], f32, name="misc", tag="misc")
            nc.tensor.matmul(rh[:], rperm[:], acc[:], start=True, stop=True)
            # t1 and the final add are SBUF-only -> gpsimd (otherwise idle);
            # t2 reads PSUM so it must stay on the DVE.
            t1 = tmp.tile([128, SB], f32, name="rope_t1")
            nc.gpsimd.tensor_tensor(out=t1[:], in0=acc[:],
                                    in1=cosT_t[:, j * SB:(j + 1) * SB], op=mult)
            t2 = tmp.tile([128, SB], f32, name="rope_t2")
            nc.vector.tensor_tensor(out=t2[:], in0=rh[:],
                                    in1=sinT_t[:, j * SB:(j + 1) * SB], op=mult)
            nc.gpsimd.tensor_tensor(out=dst_slice, in0=t1[:], in1=t2[:], op=add)

        def emit_oproj(j):
            """Partial o_proj for the 4 s-tiles of s-block j."""
            for st in range(4):
                s0 = (4 * j + st) * 128
                for ho in range(NHO):
                    opsp = pj_ps.tile([128, 2, SB], f32, name="o_psum", tag="pj")
                    ops = opsp[:, 0, :]
                    for ch in range(QH):
                        nc.tensor.matmul(
                            ops,
                            ctxT_all[:, ch * S + s0:ch * S + s0 + 128],
                            wo_t[ch][:, ho * SB:(ho + 1) * SB],
                            start=(ch == 0), stop=(ch == QH - 1),
                        )
                    osb = tmp.tile([128, SB], bf16, name="o_sb", bufs=4)
                    if ho % 2 == 0:
                        nc.vector.tensor_copy(osb[:], ops)
                    else:
                        nc.scalar.copy(osb[:], ops)
                    nc.sync.dma_start(out_d[s0:s0 + 128, ho * SB:(ho + 1) * SB],
                                      osb[:])

        # Warm the PE clock (HAM) during the initial weight/hs DMA wait:
        # ~45 throwaway matmuls on whatever is in SBUF. Results are never
        # read; the bank is overwritten with start=True afterwards.
        for w in range(45):
            wps = misc_ps.tile([128, SB], f32, name="warm_ps", tag="misc")
            nc.tensor.matmul(wps[:], ident[:], cosT_t[:, :SB],
                             start=True, stop=True)

        for j in range(NSB):
            sl = slice(j * SB, (j + 1) * SB)

            # ---- projections for s-block j ----
            # 6 targets (4 q heads, k, v) accumulate into SBUF f32 tiles;
            # PSUM holds only a 2-deep rotation of group partial sums.
            q_acc = [acc_pool.tile([128, SB], f32, name=f"q_acc{h}")
                     for h in range(QH)]
            k_acc = acc_pool.tile([128, SB], f32, name="k_acc")
            v_acc = acc_pool.tile([128, SB], f32, name="v_acc")
            def wq_sel(ih, h):
                t, ig = _wslice(wq_t, ih)
                return t[:, ig, h * 128:(h + 1) * 128]

            targets = (
                [(q_acc[h], lambda ih, h=h: wq_sel(ih, h)) for h in range(QH)]
                + [(k_acc, lambda ih: _wslice(wk_t, ih)[0][:, _wslice(wk_t, ih)[1], :]),
                   (v_acc, lambda ih: _wslice(wv_t, ih)[0][:, _wslice(wv_t, ih)[1], :])])
            # Accumulation groups (PSUM depth) vs hs DMA subtiles (<=8 chunks)
            # are decoupled: j=0 streams finely behind the weight DMAs, j>0
            # uses deep 16-chunk accumulation to halve the DVE add traffic.
            group_sizes = WGROUPS if j == 0 else [16, 16]
            ih0 = 0
            for g, gsz in enumerate(group_sizes):
                if j == 0:
                    # wq (the big one) on the second HWDGE channel (ACT
                    # queue); wk/wv ride the sync queue with hs
                    nc.scalar.dma_start(wq_t[g][:], wq_r[:, ih0:ih0 + gsz, :])
                    nc.sync.dma_start(wk_t[g][:], wk_r[:, ih0:ih0 + gsz, :])
                    nc.sync.dma_start(wv_t[g][:], wv_r[:, ih0:ih0 + gsz, :])
                hs_ts = []
                for sb0 in range(0, gsz, 8):
                    ssz = min(8, gsz - sb0)
                    hs_t = hs_pool.tile([128, ssz, SB], bf16, name="hs_t")
                    nc.sync.dma_start(hs_t[:],
                                      hs_r[:, ih0 + sb0:ih0 + sb0 + ssz, sl])
                    hs_ts.append((sb0, ssz, hs_t))
                for acc, wsel in targets:
                    psp = pj_ps.tile([128, 2, SB], f32, name="proj_ps", tag="pj")
                    ps = psp[:, 0, :]
                    for sb0, ssz, hs_t in hs_ts:
                        for ig in range(ssz):
                            i_g = sb0 + ig
                            nc.tensor.matmul(ps, wsel(ih0 + i_g), hs_t[:, ig, :],
                                             start=(i_g == 0),
                                             stop=(i_g == gsz - 1))
                    if g == 0:
                        nc.vector.tensor_copy(acc[:], ps)
                    else:
                        nc.vector.tensor_tensor(out=acc[:], in0=acc[:], in1=ps,
                                                op=add)
                ih0 += gsz
                if j == 0 and g == 0:
                    load_consts()

            # ---- o_proj of the previous block (keeps PE busy during RoPE) ----
            if j == 0:
                for ch in range(QH):
                    eng = nc.scalar if ch % 2 == 0 else nc.sync
                    eng.dma_start(wo_t[ch][:], wo_r[:, ch, :])
            else:
                emit_oproj(j - 1)

            # ---- RoPE: head 0 first (it gates the first QK), then k, v ----
            rope_emit(q_acc[0], qT_all[:, 0 * S + j * SB:0 * S + (j + 1) * SB], j)
            rope_emit(k_acc, kT_all[:, sl], j)
            vT_sb = tmp.tile([128, SB], bf16, name="vT_sb")
            nc.gpsimd.tensor_copy(vT_sb[:], v_acc[:])
            for sc in range(4):
                trp = misc_ps.tile([128, 128], bf16, name="tr_ps", tag="misc")
                nc.tensor.transpose(trp[:], vT_sb[:, sc * 128:(sc + 1) * 128],
                                    ident[:])
                s0 = (4 * j + sc) * 128
                nc.vector.tensor_copy(v_all[:, s0:s0 + 128], trp[:])
            for h in range(1, QH):
                rope_emit(q_acc[h],
                          qT_all[:, h * S + j * SB:h * S + (j + 1) * SB], j)

            # ---- attention for the 4 q heads of this core, q-block j ----
            # Stage A (per head): scores -> exp -> AV + l accumulation.
            # Stage B (normalize ctx) is deferred one head so the PE never
            # blocks on the reciprocal chain of the head it just finished.
            nk = 4 * (j + 1)          # causal: k-chunks 0..4j+3
            pending = []              # (h, outp, rc)

            def stage_b(h, outp, rc):
                bcp = pj_ps.tile([128, 2, SB], f32, name="bc_ps", tag="pj")
                bc = bcp[:, 0, :]
                nc.tensor.matmul(bc, ones_row[:], rc[:], start=True, stop=True)
                bcs = tmp.tile([128, SB], f32, name="bc_sb")
                nc.vector.tensor_copy(bcs[:], bc)
                nc.vector.tensor_tensor(
                    out=ctxT_all[:, h * S + j * SB:h * S + (j + 1) * SB],
                    in0=outp[:], in1=bcs[:], op=mult)

            for h in range(QH):
                q_rhs = qT_all[:, h * S + j * SB:h * S + (j + 1) * SB]
                outp = att_ps.tile([128, SB], f32, name="outT_ps")
                lp = misc_ps.tile([1, SB], f32, name="l_ps", tag="misc")
                npair = nk // 2

                def qk_pair(t):
                    sp = pj_ps.tile([128, 2, SB], f32, name="s_psum", tag="pj")
                    for u in range(2):
                        nc.tensor.matmul(sp[:, u, :],
                                         kT_all[:, (2 * t + u) * 128:
                                                (2 * t + u + 1) * 128],
                                         q_rhs, start=True, stop=True)
                    return sp

                sp = qk_pair(0)
                for t in range(npair):
                    pt = p_pool.tile([128, 2, SB], bf16, name="p_t")
                    nc.scalar.activation(pt[:], sp[:], Exp, scale=SCALE)
                    for u in range(2):
                        i = 2 * t + u
                        if i >= 4 * j:
                            nc.vector.tensor_tensor(
                                out=pt[:, u, :], in0=pt[:, u, :],
                                in1=masks[:, i - 4 * j, :], op=mult)
                    if t + 1 < npair:
                        sp = qk_pair(t + 1)
                    for u in range(2):
                        i = 2 * t + u
                        first, last = i == 0, i == nk - 1
                        nc.tensor.matmul(outp[:],
                                         v_all[:, i * 128:(i + 1) * 128],
                                         pt[:, u, :], start=first, stop=last)
                    for u in range(2):
                        i = 2 * t + u
                        first, last = i == 0, i == nk - 1
                        nc.tensor.matmul(lp[:], ones_col[:], pt[:, u, :],
                                         start=first, stop=last)

                rc = tmp.tile([1, SB], f32, name="recip")
                nc.vector.reciprocal_approx_fast(out=rc[:], in_=lp[:])
                pending.append((h, outp, rc))
                if len(pending) > 1:
                    stage_b(*pending.pop(0))
            while pending:
                stage_b(*pending.pop(0))

        emit_oproj(NSB - 1)


def _build():
    if "nc" in _cache:
        return _cache["nc"]
    import concourse.bacc as bacc
    import concourse.bass as bass
    import concourse.tile as tile
    from concourse import mybir

    nc = bacc.Bacc("TRN2", target_bir_lowering=False, debug=False,
                   enable_asserts=False, num_devices=NCORES)
    with tile.TileContext(nc) as tc:
        _emit_program(nc, tc, mybir, bass)
    nc.compile()
    _cache["nc"] = nc
    return nc


def _host_consts():
    rperm = np.zeros((128, 128), np.float32)
    half = D // 2
    for d in range(half):
        rperm[d + half, d] = -1.0      # out[d] = -q[d+64]  (d < 64)
        rperm[d, d + half] = 1.0       # out[d+64] = q[d]
    ident = np.eye(128, dtype=BF16)
    ones_col = np.ones((128, 1), BF16)
    ones_row = np.ones((1, 128), np.float32)
    nmr = SB // 128
    masks = np.zeros((nmr, 128, SB), np.float32)
    kk = np.arange(128)[:, None]
    qq = np.arange(SB)[None, :]
    for r in range(nmr):
        masks[r] = (128 * r + kk <= qq).astype(np.float32)
    return {
        "rperm": rperm,
        "ident": ident,
        "ones_col": ones_col,
        "ones_row": ones_row,
        "masks": masks.astype(BF16),
    }


def make_in_maps(hidden_states, cos, sin, Wq, Wk, Wv, Wo):
    hs = np.asarray(hidden_states, np.float32).reshape(S, H)
    hsT = np.ascontiguousarray(hs.T).astype(BF16)
    cosT = np.ascontiguousarray(np.asarray(cos, np.float32).reshape(S, D).T).astype(BF16)
    sinT = np.ascontiguousarray(np.asarray(sin, np.float32).reshape(S, D).T).astype(BF16)
    Wq = np.asarray(Wq, np.float32)
    Wk = np.asarray(Wk, np.float32)
    Wv = np.asarray(Wv, np.float32)
    Wo = np.asarray(Wo, np.float32)
    consts = _host_consts()
    in_maps = []
    for c in range(NCORES):
        m = dict(consts)
        m["hsT"] = hsT
        m["cosT"] = cosT
        m["sinT"] = sinT
        m["wqT"] = np.ascontiguousarray(Wq[c * DC:(c + 1) * DC].T).astype(BF16)
        m["wkT"] = np.ascontiguousarray(Wk[c * D:(c + 1) * D].T).astype(BF16)
        m["wvT"] = np.ascontiguousarray(Wv[c * D:(c + 1) * D].T).astype(BF16)
        m["woT"] = np.ascontiguousarray(Wo[:, c * DC:(c + 1) * DC].T).astype(BF16)
        in_maps.append(m)
    return in_maps


def run_spmd(in_maps, trace=False):
    nc = _build()
    from concourse import bass_utils
    if trace:
        _install_profile_shim()
    return bass_utils.run_bass_kernel_spmd(
        nc, in_maps, core_ids=list(range(NCORES)), trace=trace)


def _install_profile_shim():
    """antenv.axon_hooks is missing from this image; inject an equivalent so
    run_bass_kernel_spmd(trace=True) can reach the libaxon NTFF profiler."""
    import types
    if "antenv.axon_hooks" in sys.modules:
        return
    mod = types.ModuleType("antenv.axon_hooks")
    state = {"hook": None}
    mod.set_axon_ntff_profile_hook = lambda h: state.__setitem__("hook", h)
    mod.get_axon_ntff_profile_hook = lambda: state["hook"]
    sys.modules["antenv.axon_hooks"] = mod
    try:
        import antenv
        antenv.axon_hooks = mod
        if "/root/.axon_site" not in sys.path:
            sys.path.insert(0, "/root/.axon_site")
        from trn_agent_boot.trn_boot import _ntff_profile_via_ctypes
        hook = _ntff_profile_via_ctypes("/opt/axon/libaxon_pjrt.so")
        if hook is not None:
            mod.set_axon_ntff_profile_hook(hook)
    except Exception:
        pass


def kernel(hidden_states, cos, sin, Wq, Wk, Wv, Wo):
    in_maps = make_in_maps(hidden_states, cos, sin, Wq, Wk, Wv, Wo)
    res = run_spmd(in_maps)
    total = np.zeros((S, H), np.float64)
    for c in range(NCORES):
        total += np.asarray(res.results[c]["out"], np.float64)
    return total.astype(np.float32).reshape(1, S, H)


if __name__ == "__main__":
    rng = np.random.default_rng(0)
    ins = {
        "hidden_states": rng.standard_normal((1, S, H)).astype(np.float32),
        "cos": rng.random((1, S, D)).astype(np.float32),
        "sin": rng.random((1, S, D)).astype(np.float32),
        "Wq": (rng.standard_normal((NQH * D, H)) * 0.02).astype(np.float32),
        "Wk": (rng.standard_normal((NKVH * D, H)) * 0.02).astype(np.float32),
        "Wv": (rng.standard_normal((NKVH * D, H)) * 0.02).astype(np.float32),
        "Wo": (rng.standard_normal((H, NQH * D)) * 0.02).astype(np.float32),
    }
    out = kernel(**ins)
    print("out", out.shape, out.dtype, np.abs(out).mean())


# revision 46
# speedup vs baseline: 1.0334x; 1.0334x over previous
"""Llama attention layer (GQA + RoPE + causal softmax + o_proj) on 8 Trainium2
NeuronCores.

Sharding: tensor-parallel across heads. Core c owns q-heads [4c..4c+3] and
kv-head c (GQA group), computes q/k/v projections for those heads, RoPE,
causal attention, and a partial o_proj against its 512 columns of Wo. The
8 partial [S, H] outputs are summed at the unshard step (the "all-reduce
after o_proj" of the TP recipe).

Dataflow on each core is transpose-free:
  - hsT (host-pretransposed [H, S] bf16) feeds the projections with the
    contraction dim H on partitions.
  - q, k are produced transposed: qT/kT = [d, s]; RoPE is applied in this
    layout (rotate_half becomes a 128x128 signed-permutation matmul).
  - scores are computed transposed, scoresT = [k, q] = kT.T-slice @ qT,
    so softmax'd probsT feed the AV matmul directly as rhs and v ([s, d],
    from a PE transpose of vT) as lhsT, producing ctxT = [d, s].
  - ctxT slices are exactly the lhsT the o_proj matmul needs.
  - softmax skips the running max (scores are O(10) here, exp is safe in
    fp32), the denominator l = sum_k exp comes from a ones-vector matmul,
    and 1/l is broadcast along partitions with a rank-1 PE matmul and
    folded into the ctx eviction.

All matmuls run in bf16 with fp32 PSUM accumulation (fp32 matmuls cost 2
cycles/row on TRN2); RoPE and softmax statistics stay in fp32.
"""

import os
import sys

import numpy as np

for _p in ("/opt/trn_rl_repo", "/root/.axon_site/_ro/trn_rl_repo"):
    if os.path.isdir(_p) and _p not in sys.path:
        sys.path.insert(0, _p)

import ml_dtypes

BF16 = ml_dtypes.bfloat16

# Problem shape (hardcoded per contract).
S = 2048          # sequence length
H = 4096          # hidden size
D = 128           # head dim
NQH = 32          # total q heads
NKVH = 8          # total kv heads
NCORES = 8
QH = NQH // NCORES          # q heads per core (4)
DC = QH * D                 # ctx dims per core (512)
SB = 512                    # s-block (matmul free dim)
NSB = S // SB               # 4 s-blocks
NHCH = H // 128             # 32 contraction chunks for projections
NHO = H // SB               # 8 output column blocks for o_proj
SCALE = float(D) ** -0.5

_cache: dict = {}


def _emit_program(nc, tc, mybir, bass):
    """Emit the per-core Tile program (SPMD: identical on all 8 cores)."""
    from contextlib import ExitStack

    f32 = mybir.dt.float32
    bf16 = mybir.dt.bfloat16
    Exp = mybir.ActivationFunctionType.Exp
    mult = mybir.AluOpType.mult
    add = mybir.AluOpType.add

    # DRAM I/O access patterns
    hs_r = nc.dram_tensor("hsT", [H, S], bf16, kind="ExternalInput").ap() \
        .rearrange("(n p) s -> p n s", p=128)                    # [128, 32, S]
    cosT = nc.dram_tensor("cosT", [D, S], bf16, kind="ExternalInput").ap()
    sinT = nc.dram_tensor("sinT", [D, S], bf16, kind="ExternalInput").ap()
    wq_r = nc.dram_tensor("wqT", [H, DC], bf16, kind="ExternalInput").ap() \
        .rearrange("(n p) m -> p n m", p=128)                    # [128, 32, 512]
    wk_r = nc.dram_tensor("wkT", [H, D], bf16, kind="ExternalInput").ap() \
        .rearrange("(n p) m -> p n m", p=128)                    # [128, 32, 128]
    wv_r = nc.dram_tensor("wvT", [H, D], bf16, kind="ExternalInput").ap() \
        .rearrange("(n p) m -> p n m", p=128)
    wo_r = nc.dram_tensor("woT", [DC, H], bf16, kind="ExternalInput").ap() \
        .rearrange("(n p) m -> p n m", p=128)                    # [128, 4, 4096]
    rperm_d = nc.dram_tensor("rperm", [128, 128], f32, kind="ExternalInput").ap()
    ident_d = nc.dram_tensor("ident", [128, 128], bf16, kind="ExternalInput").ap()
    onesc_d = nc.dram_tensor("ones_col", [128, 1], bf16, kind="ExternalInput").ap()
    onesr_d = nc.dram_tensor("ones_row", [1, 128], f32, kind="ExternalInput").ap()
    masks_r = nc.dram_tensor("masks", [SB // 128, 128, SB], bf16,
                             kind="ExternalInput").ap().rearrange("r p q -> p r q")
    out_d = nc.dram_tensor("out", [S, H], bf16, kind="ExternalOutput").ap()

    ctx = ExitStack()
    with ctx:
        consts = ctx.enter_context(tc.tile_pool(name="consts", bufs=1))
        persist = ctx.enter_context(tc.tile_pool(name="persist", bufs=1))
        wpool = ctx.enter_context(tc.tile_pool(name="wpool", bufs=1))
        hs_pool = ctx.enter_context(tc.tile_pool(name="hs_pool", bufs=3))
        acc_pool = ctx.enter_context(tc.tile_pool(name="acc_pool", bufs=1))
        tmp = ctx.enter_context(tc.tile_pool(name="tmp", bufs=2))
        p_pool = ctx.enter_context(tc.tile_pool(name="p_pool", bufs=5))
        # PSUM budget (8 banks): pj tag holds [128, 2, SB] 2-bank slots x2
        # (proj groups + o_proj + paired score tiles), outT 2, misc 2
        # (rope/transpose/bcast + the l accumulator).
        pj_ps = ctx.enter_context(tc.tile_pool(name="pj_ps", bufs=2, space="PSUM"))
        misc_ps = ctx.enter_context(tc.tile_pool(name="misc_ps", bufs=2, space="PSUM"))
        att_ps = ctx.enter_context(tc.tile_pool(name="att_ps", bufs=2, space="PSUM"))

        # Constants (tiles now; DMAs are emitted lazily after the first
        # projection group so the critical first weight/hs loads go first)
        rperm = consts.tile([128, 128], f32, name="rperm")
        ident = consts.tile([128, 128], bf16, name="ident")
        ones_col = consts.tile([128, 1], bf16, name="ones_col")
        ones_row = consts.tile([1, 128], f32, name="ones_row")
        masks = consts.tile([128, SB // 128, SB], bf16, name="masks")
        cosT_t = consts.tile([D, S], bf16, name="cosT")
        sinT_t = consts.tile([D, S], bf16, name="sinT")

        def load_consts():
            nc.sync.dma_start(rperm[:], rperm_d)
            nc.sync.dma_start(ident[:], ident_d)
            nc.sync.dma_start(ones_col[:], onesc_d)
            nc.sync.dma_start(ones_row[:], onesr_d)
            nc.sync.dma_start(masks[:], masks_r)
            nc.sync.dma_start(cosT_t[:], cosT)
            nc.sync.dma_start(sinT_t[:], sinT)

        # Persistent activations
        qT_all = persist.tile([128, QH * S], bf16, name="qT_all")   # per head [d, s]
        kT_all = persist.tile([128, S], bf16, name="kT_all")        # [d, s]
        v_all = persist.tile([128, S], bf16, name="v_all")          # 16 x [s128, d128]
        ctxT_all = persist.tile([128, QH * S], bf16, name="ctxT_all")

        # Weight tiles, loaded in group-granular DMAs pipelined with the
        # first projection pass. GROUPS lists the H-chunk count per PSUM
        # accumulation group; the first groups are small so the very first
        # matmuls wait on less DMA.
        WGROUPS = [2, 2, 4, 8, 8, 8]         # weight DMA granularity (j=0)
        WOFF = [sum(WGROUPS[:g]) for g in range(len(WGROUPS))]
        wq_t = [wpool.tile([128, gsz, DC], bf16, name=f"wq{g}")
                for g, gsz in enumerate(WGROUPS)]
        wk_t = [wpool.tile([128, gsz, D], bf16, name=f"wk{g}")
                for g, gsz in enumerate(WGROUPS)]
        wv_t = [wpool.tile([128, gsz, D], bf16, name=f"wv{g}")
                for g, gsz in enumerate(WGROUPS)]
        wo_t = [wpool.tile([128, H], bf16, name=f"wo{ch}") for ch in range(QH)]

        def _wslice(tiles, ih):
            for g in range(len(WGROUPS) - 1, -1, -1):
                if ih >= WOFF[g]:
                    return tiles[g], ih - WOFF[g]
            raise AssertionError

        def rope_emit(acc, dst_slice, j):
            """dst(bf16) = acc*cosT + rotate_half(acc)*sinT for s-block j.

            acc is the SBUF f32 projection accumulator; rotate_half is a
            signed 128x128 permutation done on the PE."""
            rh = misc_ps.tile([128, SB], f32, name="misc", tag="misc")
            nc.tensor.matmul(rh[:], rperm[:], acc[:], start=True, stop=True)
            # t1 and the final add are SBUF-only -> gpsimd (otherwise idle);
            # t2 reads PSUM so it must stay on the DVE.
            t1 = tmp.tile([128, SB], f32, name="rope_t1")
            nc.gpsimd.tensor_tensor(out=t1[:], in0=acc[:],
                                    in1=cosT_t[:, j * SB:(j + 1) * SB], op=mult)
            t2 = tmp.tile([128, SB], f32, name="rope_t2")
            nc.vector.tensor_tensor(out=t2[:], in0=rh[:],
                                    in1=sinT_t[:, j * SB:(j + 1) * SB], op=mult)
            nc.gpsimd.tensor_tensor(out=dst_slice, in0=t1[:], in1=t2[:], op=add)

        def emit_oproj(j):
            """Partial o_proj for the 4 s-tiles of s-block j."""
            for st in range(4):
                s0 = (4 * j + st) * 128
                for ho in range(NHO):
                    opsp = pj_ps.tile([128, 2, SB], f32, name="o_psum", tag="pj")
                    ops = opsp[:, 0, :]
                    for ch in range(QH):
                        nc.tensor.matmul(
                            ops,
                            ctxT_all[:, ch * S + s0:ch * S + s0 + 128],
                            wo_t[ch][:, ho * SB:(ho + 1) * SB],
                            start=(ch == 0), stop=(ch == QH - 1),
                        )
                    osb = tmp.tile([128, SB], bf16, name="o_sb", bufs=4)
                    if ho % 2 == 0:
                        nc.vector.tensor_copy(osb[:], ops)
                    else:
                        nc.scalar.copy(osb[:], ops)
                    nc.sync.dma_start(out_d[s0:s0 + 128, ho * SB:(ho + 1) * SB],
                                      osb[:])

        # Warm the PE clock (HAM) during the initial weight/hs DMA wait:
        # ~45 throwaway matmuls on whatever is in SBUF. Results are never
        # read; the bank is overwritten with start=True afterwards.
        for w in range(45):
            wps = misc_ps.tile([128, SB], f32, name="warm_ps", tag="misc")
            nc.tensor.matmul(wps[:], ident[:], cosT_t[:, :SB],
                             start=True, stop=True)

        for j in range(NSB):
            sl = slice(j * SB, (j + 1) * SB)

            # ---- projections for s-block j ----
            # 6 targets (4 q heads, k, v) accumulate into SBUF f32 tiles;
            # PSUM holds only a 2-deep rotation of group partial sums.
            q_acc = [acc_pool.tile([128, SB], f32, name=f"q_acc{h}")
                     for h in range(QH)]
            k_acc = acc_pool.tile([128, SB], f32, name="k_acc")
            v_acc = acc_pool.tile([128, SB], f32, name="v_acc")
            def wq_sel(ih, h):
                t, ig = _wslice(wq_t, ih)
                return t[:, ig, h * 128:(h + 1) * 128]

            targets = (
                [(q_acc[h], lambda ih, h=h: wq_sel(ih, h)) for h in range(QH)]
                + [(k_acc, lambda ih: _wslice(wk_t, ih)[0][:, _wslice(wk_t, ih)[1], :]),
                   (v_acc, lambda ih: _wslice(wv_t, ih)[0][:, _wslice(wv_t, ih)[1], :])])
            # Accumulation groups (PSUM depth) vs hs DMA subtiles (<=8 chunks)
            # are decoupled: j=0 streams finely behind the weight DMAs, j>0
            # uses deep 16-chunk accumulation to halve the DVE add traffic.
            group_sizes = WGROUPS if j == 0 else [16, 16]
            ih0 = 0
            for g, gsz in enumerate(group_sizes):
                if j == 0:
                    # second HWDGE channel (ACT queue), parallel to the
                    # hs stream on the sync queue
                    nc.scalar.dma_start(wq_t[g][:], wq_r[:, ih0:ih0 + gsz, :])
                    nc.scalar.dma_start(wk_t[g][:], wk_r[:, ih0:ih0 + gsz, :])
                    nc.scalar.dma_start(wv_t[g][:], wv_r[:, ih0:ih0 + gsz, :])
                hs_ts = []
                for sb0 in range(0, gsz, 8):
                    ssz = min(8, gsz - sb0)
                    hs_t = hs_pool.tile([128, ssz, SB], bf16, name="hs_t")
                    nc.sync.dma_start(hs_t[:],
                                      hs_r[:, ih0 + sb0:ih0 + sb0 + ssz, sl])
                    hs_ts.append((sb0, ssz, hs_t))
                for acc, wsel in targets:
                    psp = pj_ps.tile([128, 2, SB], f32, name="proj_ps", tag="pj")
                    ps = psp[:, 0, :]
                    for sb0, ssz, hs_t in hs_ts:
                        for ig in range(ssz):
                            i_g = sb0 + ig
                            nc.tensor.matmul(ps, wsel(ih0 + i_g), hs_t[:, ig, :],
                                             start=(i_g == 0),
                                             stop=(i_g == gsz - 1))
                    if g == 0:
                        nc.vector.tensor_copy(acc[:], ps)
                    else:
                        nc.vector.tensor_tensor(out=acc[:], in0=acc[:], in1=ps,
                                                op=add)
                ih0 += gsz
                if j == 0 and g == 0:
                    load_consts()

            # ---- o_proj of the previous block (keeps PE busy during RoPE) ----
            if j == 0:
                for ch in range(QH):
                    nc.scalar.dma_start(wo_t[ch][:], wo_r[:, ch, :])
            else:
                emit_oproj(j - 1)

            # ---- RoPE: head 0 first (it gates the first QK), then k, v ----
            rope_emit(q_acc[0], qT_all[:, 0 * S + j * SB:0 * S + (j + 1) * SB], j)
            rope_emit(k_acc, kT_all[:, sl], j)
            vT_sb = tmp.tile([128, SB], bf16, name="vT_sb")
            nc.gpsimd.tensor_copy(vT_sb[:], v_acc[:])
            for sc in range(4):
                trp = misc_ps.tile([128, 128], bf16, name="tr_ps", tag="misc")
                nc.tensor.transpose(trp[:], vT_sb[:, sc * 128:(sc + 1) * 128],
                                    ident[:])
                s0 = (4 * j + sc) * 128
                nc.vector.tensor_copy(v_all[:, s0:s0 + 128], trp[:])
            for h in range(1, QH):
                rope_emit(q_acc[h],
                          qT_all[:, h * S + j * SB:h * S + (j + 1) * SB], j)

            # ---- attention for the 4 q heads of this core, q-block j ----
            # Stage A (per head): scores -> exp -> AV + l accumulation.
            # Stage B (normalize ctx) is deferred one head so the PE never
            # blocks on the reciprocal chain of the head it just finished.
            nk = 4 * (j + 1)          # causal: k-chunks 0..4j+3
            pending = []              # (h, outp, rc)

            def stage_b(h, outp, rc):
                bcp = pj_ps.tile([128, 2, SB], f32, name="bc_ps", tag="pj")
                bc = bcp[:, 0, :]
                nc.tensor.matmul(bc, ones_row[:], rc[:], start=True, stop=True)
                bcs = tmp.tile([128, SB], f32, name="bc_sb")
                nc.vector.tensor_copy(bcs[:], bc)
                nc.vector.tensor_tensor(
                    out=ctxT_all[:, h * S + j * SB:h * S + (j + 1) * SB],
                    in0=outp[:], in1=bcs[:], op=mult)

            for h in range(QH):
                q_rhs = qT_all[:, h * S + j * SB:h * S + (j + 1) * SB]
                outp = att_ps.tile([128, SB], f32, name="outT_ps")
                lp = misc_ps.tile([1, SB], f32, name="l_ps", tag="misc")
                npair = nk // 2

                def qk_pair(t):
                    sp = pj_ps.tile([128, 2, SB], f32, name="s_psum", tag="pj")
                    for u in range(2):
                        nc.tensor.matmul(sp[:, u, :],
                                         kT_all[:, (2 * t + u) * 128:
                                                (2 * t + u + 1) * 128],
                                         q_rhs, start=True, stop=True)
                    return sp

                sp = qk_pair(0)
                for t in range(npair):
                    pt = p_pool.tile([128, 2, SB], bf16, name="p_t")
                    nc.scalar.activation(pt[:], sp[:], Exp, scale=SCALE)
                    for u in range(2):
                        i = 2 * t + u
                        if i >= 4 * j:
                            nc.vector.tensor_tensor(
                                out=pt[:, u, :], in0=pt[:, u, :],
                                in1=masks[:, i - 4 * j, :], op=mult)
                    if t + 1 < npair:
                        sp = qk_pair(t + 1)
                    for u in range(2):
                        i = 2 * t + u
                        first, last = i == 0, i == nk - 1
                        nc.tensor.matmul(outp[:],
                                         v_all[:, i * 128:(i + 1) * 128],
                                         pt[:, u, :], start=first, stop=last)
                    for u in range(2):
                        i = 2 * t + u
                        first, last = i == 0, i == nk - 1
                        nc.tensor.matmul(lp[:], ones_col[:], pt[:, u, :],
                                         start=first, stop=last)

                rc = tmp.tile([1, SB], f32, name="recip")
                nc.vector.reciprocal_approx_fast(out=rc[:], in_=lp[:])
                pending.append((h, outp, rc))
                if len(pending) > 1:
                    stage_b(*pending.pop(0))
            while pending:
                stage_b(*pending.pop(0))

        emit_oproj(NSB - 1)


def _build():
    if "nc" in _cache:
        return _cache["nc"]
    import concourse.bacc as bacc
    import concourse.bass as bass
    import concourse.tile as tile
    from concourse import mybir

    nc = bacc.Bacc("TRN2", target_bir_lowering=False, debug=False,
                   enable_asserts=False, num_devices=NCORES)
    with tile.TileContext(nc) as tc:
        _emit_program(nc, tc, mybir, bass)
    nc.compile()
    _cache["nc"] = nc
    return nc


def _host_consts():
    rperm = np.zeros((128, 128), np.float32)
    half = D // 2
    for d in range(half):
        rperm[d + half, d] = -1.0      # out[d] = -q[d+64]  (d < 64)
        rperm[d, d + half] = 1.0       # out[d+64] = q[d]
    ident = np.eye(128, dtype=BF16)
    ones_col = np.ones((128, 1), BF16)
    ones_row = np.ones((1, 128), np.float32)
    nmr = SB // 128
    masks = np.zeros((nmr, 128, SB), np.float32)
    kk = np.arange(128)[:, None]
    qq = np.arange(SB)[None, :]
    for r in range(nmr):
        masks[r] = (128 * r + kk <= qq).astype(np.float32)
    return {
        "rperm": rperm,
        "ident": ident,
        "ones_col": ones_col,
        "ones_row": ones_row,
        "masks": masks.astype(BF16),
    }


def make_in_maps(hidden_states, cos, sin, Wq, Wk, Wv, Wo):
    hs = np.asarray(hidden_states, np.float32).reshape(S, H)
    hsT = np.ascontiguousarray(hs.T).astype(BF16)
    cosT = np.ascontiguousarray(np.asarray(cos, np.float32).reshape(S, D).T).astype(BF16)
    sinT = np.ascontiguousarray(np.asarray(sin, np.float32).reshape(S, D).T).astype(BF16)
    Wq = np.asarray(Wq, np.float32)
    Wk = np.asarray(Wk, np.float32)
    Wv = np.asarray(Wv, np.float32)
    Wo = np.asarray(Wo, np.float32)
    consts = _host_consts()
    in_maps = []
    for c in range(NCORES):
        m = dict(consts)
        m["hsT"] = hsT
        m["cosT"] = cosT
        m["sinT"] = sinT
        m["wqT"] = np.ascontiguousarray(Wq[c * DC:(c + 1) * DC].T).astype(BF16)
        m["wkT"] = np.ascontiguousarray(Wk[c * D:(c + 1) * D].T).astype(BF16)
        m["wvT"] = np.ascontiguousarray(Wv[c * D:(c + 1) * D].T).astype(BF16)
        m["woT"] = np.ascontiguousarray(Wo[:, c * DC:(c + 1) * DC].T).astype(BF16)
        in_maps.append(m)
    return in_maps


def run_spmd(in_maps, trace=False):
    nc = _build()
    from concourse import bass_utils
    if trace:
        _install_profile_shim()
    return bass_utils.run_bass_kernel_spmd(
        nc, in_maps, core_ids=list(range(NCORES)), trace=trace)


def _install_profile_shim():
    """antenv.axon_hooks is missing from this image; inject an equivalent so
    run_bass_kernel_spmd(trace=True) can reach the libaxon NTFF profiler."""
    import types
    if "antenv.axon_hooks" in sys.modules:
        return
    mod = types.ModuleType("antenv.axon_hooks")
    state = {"hook": None}
    mod.set_axon_ntff_profile_hook = lambda h: state.__setitem__("hook", h)
    mod.get_axon_ntff_profile_hook = lambda: state["hook"]
    sys.modules["antenv.axon_hooks"] = mod
    try:
        import antenv
        antenv.axon_hooks = mod
        if "/root/.axon_site" not in sys.path:
            sys.path.insert(0, "/root/.axon_site")
        from trn_agent_boot.trn_boot import _ntff_profile_via_ctypes
        hook = _ntff_profile_via_ctypes("/opt/axon/libaxon_pjrt.so")
        if hook is not None:
            mod.set_axon_ntff_profile_hook(hook)
    except Exception:
        pass


def kernel(hidden_states, cos, sin, Wq, Wk, Wv, Wo):
    in_maps = make_in_maps(hidden_states, cos, sin, Wq, Wk, Wv, Wo)
    res = run_spmd(in_maps)
    total = np.zeros((S, H), np.float64)
    for c in range(NCORES):
        total += np.asarray(res.results[c]["out"], np.float64)
    return total.astype(np.float32).reshape(1, S, H)


if __name__ == "__main__":
    rng = np.random.default_rng(0)
    ins = {
        "hidden_states": rng.standard_normal((1, S, H)).astype(np.float32),
        "cos": rng.random((1, S, D)).astype(np.float32),
        "sin": rng.random((1, S, D)).astype(np.float32),
        "Wq": (rng.standard_normal((NQH * D, H)) * 0.02).astype(np.float32),
        "Wk": (rng.standard_normal((NKVH * D, H)) * 0.02).astype(np.float32),
        "Wv": (rng.standard_normal((NKVH * D, H)) * 0.02).astype(np.float32),
        "Wo": (rng.standard_normal((H, NQH * D)) * 0.02).astype(np.float32),
    }
    out = kernel(**ins)
    print("out", out.shape, out.dtype, np.abs(out).mean())


# revision 50
# speedup vs baseline: 1.0755x; 1.0407x over previous
"""Llama attention layer (GQA + RoPE + causal softmax + o_proj) on 8 Trainium2
NeuronCores.

Sharding: tensor-parallel across heads. Core c owns q-heads [4c..4c+3] and
kv-head c (GQA group), computes q/k/v projections for those heads, RoPE,
causal attention, and a partial o_proj against its 512 columns of Wo. The
8 partial [S, H] outputs are summed at the unshard step (the "all-reduce
after o_proj" of the TP recipe).

Dataflow on each core is transpose-free:
  - hsT (host-pretransposed [H, S] bf16) feeds the projections with the
    contraction dim H on partitions.
  - q, k are produced transposed: qT/kT = [d, s]; RoPE is applied in this
    layout (rotate_half becomes a 128x128 signed-permutation matmul).
  - scores are computed transposed, scoresT = [k, q] = kT.T-slice @ qT,
    so softmax'd probsT feed the AV matmul directly as rhs and v ([s, d],
    from a PE transpose of vT) as lhsT, producing ctxT = [d, s].
  - ctxT slices are exactly the lhsT the o_proj matmul needs.
  - softmax skips the running max (scores are O(10) here, exp is safe in
    fp32), the denominator l = sum_k exp comes from a ones-vector matmul,
    and 1/l is broadcast along partitions with a rank-1 PE matmul and
    folded into the ctx eviction.

All matmuls run in bf16 with fp32 PSUM accumulation (fp32 matmuls cost 2
cycles/row on TRN2); RoPE and softmax statistics stay in fp32.
"""

import os
import sys

import numpy as np

for _p in ("/opt/trn_rl_repo", "/root/.axon_site/_ro/trn_rl_repo"):
    if os.path.isdir(_p) and _p not in sys.path:
        sys.path.insert(0, _p)

import ml_dtypes

BF16 = ml_dtypes.bfloat16

# Problem shape (hardcoded per contract).
S = 2048          # sequence length
H = 4096          # hidden size
D = 128           # head dim
NQH = 32          # total q heads
NKVH = 8          # total kv heads
NCORES = 8
QH = NQH // NCORES          # q heads per core (4)
DC = QH * D                 # ctx dims per core (512)
SB = 512                    # s-block (matmul free dim)
NSB = S // SB               # 4 s-blocks
NHCH = H // 128             # 32 contraction chunks for projections
NHO = H // SB               # 8 output column blocks for o_proj
SCALE = float(D) ** -0.5

_cache: dict = {}


def _emit_program(nc, tc, mybir, bass):
    """Emit the per-core Tile program (SPMD: identical on all 8 cores)."""
    from contextlib import ExitStack

    f32 = mybir.dt.float32
    bf16 = mybir.dt.bfloat16
    Exp = mybir.ActivationFunctionType.Exp
    mult = mybir.AluOpType.mult
    add = mybir.AluOpType.add

    # DRAM I/O access patterns
    hs_r = nc.dram_tensor("hsT", [H, S], bf16, kind="ExternalInput").ap() \
        .rearrange("(n p) s -> p n s", p=128)                    # [128, 32, S]
    cosT = nc.dram_tensor("cosT", [D, S], bf16, kind="ExternalInput").ap()
    sinT = nc.dram_tensor("sinT", [D, S], bf16, kind="ExternalInput").ap()
    wq_r = nc.dram_tensor("wqT", [H, DC], bf16, kind="ExternalInput").ap() \
        .rearrange("(n p) m -> p n m", p=128)                    # [128, 32, 512]
    wk_r = nc.dram_tensor("wkT", [H, D], bf16, kind="ExternalInput").ap() \
        .rearrange("(n p) m -> p n m", p=128)                    # [128, 32, 128]
    wv_r = nc.dram_tensor("wvT", [H, D], bf16, kind="ExternalInput").ap() \
        .rearrange("(n p) m -> p n m", p=128)
    wo_r = nc.dram_tensor("woT", [DC, H], bf16, kind="ExternalInput").ap() \
        .rearrange("(n p) m -> p n m", p=128)                    # [128, 4, 4096]
    rperm_d = nc.dram_tensor("rperm", [128, 128], f32, kind="ExternalInput").ap()
    ident_d = nc.dram_tensor("ident", [128, 128], bf16, kind="ExternalInput").ap()
    onesc_d = nc.dram_tensor("ones_col", [128, 1], bf16, kind="ExternalInput").ap()
    onesr_d = nc.dram_tensor("ones_row", [1, 128], bf16, kind="ExternalInput").ap()
    masks_r = nc.dram_tensor("masks", [SB // 128, 128, SB], bf16,
                             kind="ExternalInput").ap().rearrange("r p q -> p r q")
    out_d = nc.dram_tensor("out", [S, H], bf16, kind="ExternalOutput").ap()

    ctx = ExitStack()
    with ctx:
        consts = ctx.enter_context(tc.tile_pool(name="consts", bufs=1))
        persist = ctx.enter_context(tc.tile_pool(name="persist", bufs=1))
        wpool = ctx.enter_context(tc.tile_pool(name="wpool", bufs=1))
        hs_pool = ctx.enter_context(tc.tile_pool(name="hs_pool", bufs=3))
        acc_pool = ctx.enter_context(tc.tile_pool(name="acc_pool", bufs=1))
        tmp = ctx.enter_context(tc.tile_pool(name="tmp", bufs=2))
        p_pool = ctx.enter_context(tc.tile_pool(name="p_pool", bufs=5))
        # PSUM budget (8 banks): pj tag holds [128, 2, SB] 2-bank slots x2
        # (proj groups + o_proj + paired score tiles), outT 2, misc 2
        # (rope/transpose/bcast + the l accumulator).
        pj_ps = ctx.enter_context(tc.tile_pool(name="pj_ps", bufs=2, space="PSUM"))
        misc_ps = ctx.enter_context(tc.tile_pool(name="misc_ps", bufs=2, space="PSUM"))
        att_ps = ctx.enter_context(tc.tile_pool(name="att_ps", bufs=2, space="PSUM"))

        # Constants (tiles now; DMAs are emitted lazily after the first
        # projection group so the critical first weight/hs loads go first)
        rperm = consts.tile([128, 128], f32, name="rperm")
        ident = consts.tile([128, 128], bf16, name="ident")
        ones_col = consts.tile([128, 1], bf16, name="ones_col")
        ones_row = consts.tile([1, 128], bf16, name="ones_row")
        masks = consts.tile([128, SB // 128, SB], bf16, name="masks")
        cosT_t = consts.tile([D, S], bf16, name="cosT")
        sinT_t = consts.tile([D, S], bf16, name="sinT")

        def load_consts():
            nc.sync.dma_start(rperm[:], rperm_d)
            nc.sync.dma_start(ident[:], ident_d)
            nc.sync.dma_start(ones_col[:], onesc_d)
            nc.sync.dma_start(ones_row[:], onesr_d)
            nc.sync.dma_start(masks[:], masks_r)
            nc.sync.dma_start(cosT_t[:], cosT)
            nc.sync.dma_start(sinT_t[:], sinT)

        # Persistent activations
        qT_all = persist.tile([128, QH * S], bf16, name="qT_all")   # per head [d, s]
        kT_all = persist.tile([128, S], bf16, name="kT_all")        # [d, s]
        v_all = persist.tile([128, S], bf16, name="v_all")          # 16 x [s128, d128]
        ctxT_all = persist.tile([128, QH * S], bf16, name="ctxT_all")

        # Weight tiles, loaded in group-granular DMAs pipelined with the
        # first projection pass. GROUPS lists the H-chunk count per PSUM
        # accumulation group; the first groups are small so the very first
        # matmuls wait on less DMA.
        WGROUPS = [2, 2, 4, 8, 8, 8]         # weight DMA granularity (j=0)
        WOFF = [sum(WGROUPS[:g]) for g in range(len(WGROUPS))]
        wq_t = [wpool.tile([128, gsz, DC], bf16, name=f"wq{g}")
                for g, gsz in enumerate(WGROUPS)]
        wk_t = [wpool.tile([128, gsz, D], bf16, name=f"wk{g}")
                for g, gsz in enumerate(WGROUPS)]
        wv_t = [wpool.tile([128, gsz, D], bf16, name=f"wv{g}")
                for g, gsz in enumerate(WGROUPS)]
        wo_t = [wpool.tile([128, H], bf16, name=f"wo{ch}") for ch in range(QH)]

        def _wslice(tiles, ih):
            for g in range(len(WGROUPS) - 1, -1, -1):
                if ih >= WOFF[g]:
                    return tiles[g], ih - WOFF[g]
            raise AssertionError

        def rope_emit(acc, dst_slice, j):
            """dst(bf16) = acc*cosT + rotate_half(acc)*sinT for s-block j.

            acc is the SBUF f32 projection accumulator; rotate_half is a
            signed 128x128 permutation done on the PE."""
            rh = misc_ps.tile([128, SB], f32, name="misc", tag="misc")
            nc.tensor.matmul(rh[:], rperm[:], acc[:], start=True, stop=True)
            # t1 and the final add are SBUF-only -> gpsimd (otherwise idle);
            # t2 reads PSUM so it must stay on the DVE.
            t1 = tmp.tile([128, SB], f32, name="rope_t1")
            nc.gpsimd.tensor_tensor(out=t1[:], in0=acc[:],
                                    in1=cosT_t[:, j * SB:(j + 1) * SB], op=mult)
            t2 = tmp.tile([128, SB], f32, name="rope_t2")
            nc.vector.tensor_tensor(out=t2[:], in0=rh[:],
                                    in1=sinT_t[:, j * SB:(j + 1) * SB], op=mult)
            nc.vector.tensor_tensor(out=dst_slice, in0=t1[:], in1=t2[:], op=add)

        def emit_oproj(j):
            """Partial o_proj for the 4 s-tiles of s-block j."""
            for st in range(4):
                s0 = (4 * j + st) * 128
                for ho in range(NHO):
                    opsp = pj_ps.tile([128, 2, SB], f32, name="o_psum", tag="pj")
                    ops = opsp[:, 0, :]
                    for ch in range(QH):
                        nc.tensor.matmul(
                            ops,
                            ctxT_all[:, ch * S + s0:ch * S + s0 + 128],
                            wo_t[ch][:, ho * SB:(ho + 1) * SB],
                            start=(ch == 0), stop=(ch == QH - 1),
                        )
                    osb = tmp.tile([128, SB], bf16, name="o_sb", bufs=4)
                    if ho % 2 == 0:
                        nc.vector.tensor_copy(osb[:], ops)
                    else:
                        nc.scalar.copy(osb[:], ops)
                    nc.sync.dma_start(out_d[s0:s0 + 128, ho * SB:(ho + 1) * SB],
                                      osb[:])

        # Warm the PE clock (HAM) during the initial weight/hs DMA wait:
        # ~45 throwaway matmuls on whatever is in SBUF. Results are never
        # read; the bank is overwritten with start=True afterwards.
        for w in range(45):
            wps = misc_ps.tile([128, SB], f32, name="warm_ps", tag="misc")
            nc.tensor.matmul(wps[:], ident[:], cosT_t[:, :SB],
                             start=True, stop=True)

        for j in range(NSB):
            sl = slice(j * SB, (j + 1) * SB)

            # ---- projections for s-block j ----
            # 6 targets (4 q heads, k, v) accumulate into SBUF f32 tiles;
            # PSUM holds only a 2-deep rotation of group partial sums.
            q_acc = [acc_pool.tile([128, SB], f32, name=f"q_acc{h}")
                     for h in range(QH)]
            k_acc = acc_pool.tile([128, SB], f32, name="k_acc")
            v_acc = acc_pool.tile([128, SB], f32, name="v_acc")
            def wq_sel(ih, h):
                t, ig = _wslice(wq_t, ih)
                return t[:, ig, h * 128:(h + 1) * 128]

            targets = (
                [(q_acc[h], lambda ih, h=h: wq_sel(ih, h)) for h in range(QH)]
                + [(k_acc, lambda ih: _wslice(wk_t, ih)[0][:, _wslice(wk_t, ih)[1], :]),
                   (v_acc, lambda ih: _wslice(wv_t, ih)[0][:, _wslice(wv_t, ih)[1], :])])
            # Accumulation groups (PSUM depth) vs hs DMA subtiles (<=8 chunks)
            # are decoupled: j=0 streams finely behind the weight DMAs, j>0
            # uses deep 16-chunk accumulation to halve the DVE add traffic.
            group_sizes = WGROUPS if j == 0 else [16, 16]
            ih0 = 0
            for g, gsz in enumerate(group_sizes):
                if j == 0:
                    # second HWDGE channel (ACT queue), parallel to the
                    # hs stream on the sync queue
                    nc.scalar.dma_start(wq_t[g][:], wq_r[:, ih0:ih0 + gsz, :])
                    nc.scalar.dma_start(wk_t[g][:], wk_r[:, ih0:ih0 + gsz, :])
                    nc.scalar.dma_start(wv_t[g][:], wv_r[:, ih0:ih0 + gsz, :])
                hs_ts = []
                for sb0 in range(0, gsz, 8):
                    ssz = min(8, gsz - sb0)
                    hs_t = hs_pool.tile([128, ssz, SB], bf16, name="hs_t")
                    nc.sync.dma_start(hs_t[:],
                                      hs_r[:, ih0 + sb0:ih0 + sb0 + ssz, sl])
                    hs_ts.append((sb0, ssz, hs_t))
                for acc, wsel in targets:
                    psp = pj_ps.tile([128, 2, SB], f32, name="proj_ps", tag="pj")
                    ps = psp[:, 0, :]
                    for sb0, ssz, hs_t in hs_ts:
                        for ig in range(ssz):
                            i_g = sb0 + ig
                            nc.tensor.matmul(ps, wsel(ih0 + i_g), hs_t[:, ig, :],
                                             start=(i_g == 0),
                                             stop=(i_g == gsz - 1))
                    if g == 0:
                        nc.vector.tensor_copy(acc[:], ps)
                    else:
                        nc.vector.tensor_tensor(out=acc[:], in0=acc[:], in1=ps,
                                                op=add)
                ih0 += gsz
                if j == 0 and g == 0:
                    load_consts()

            # ---- o_proj of the previous block (keeps PE busy during RoPE) ----
            if j == 0:
                for ch in range(QH):
                    nc.scalar.dma_start(wo_t[ch][:], wo_r[:, ch, :])
            else:
                emit_oproj(j - 1)

            # ---- RoPE: head 0 first (it gates the first QK), then k, v ----
            rope_emit(q_acc[0], qT_all[:, 0 * S + j * SB:0 * S + (j + 1) * SB], j)
            rope_emit(k_acc, kT_all[:, sl], j)
            vT_sb = tmp.tile([128, SB], bf16, name="vT_sb")
            nc.vector.tensor_copy(vT_sb[:], v_acc[:])
            for sc in range(4):
                trp = misc_ps.tile([128, 128], bf16, name="tr_ps", tag="misc")
                nc.tensor.transpose(trp[:], vT_sb[:, sc * 128:(sc + 1) * 128],
                                    ident[:])
                s0 = (4 * j + sc) * 128
                nc.vector.tensor_copy(v_all[:, s0:s0 + 128], trp[:])
            for h in range(1, QH):
                rope_emit(q_acc[h],
                          qT_all[:, h * S + j * SB:h * S + (j + 1) * SB], j)

            # ---- attention for the 4 q heads of this core, q-block j ----
            # Stage A (per head): scores -> exp -> AV + l accumulation.
            # Stage B (normalize ctx) is deferred one head so the PE never
            # blocks on the reciprocal chain of the head it just finished.
            nk = 4 * (j + 1)          # causal: k-chunks 0..4j+3
            pending = []              # (h, outp, rc)

            def stage_b(h, outp, rc):
                bcp = pj_ps.tile([128, 2, SB], f32, name="bc_ps", tag="pj")
                bc = bcp[:, 0, :]
                nc.tensor.matmul(bc, ones_row[:], rc[:], start=True, stop=True)
                bcs = tmp.tile([128, SB], f32, name="bc_sb")
                nc.vector.tensor_copy(bcs[:], bc)
                nc.vector.tensor_tensor(
                    out=ctxT_all[:, h * S + j * SB:h * S + (j + 1) * SB],
                    in0=outp[:], in1=bcs[:], op=mult)

            for h in range(QH):
                q_rhs = qT_all[:, h * S + j * SB:h * S + (j + 1) * SB]
                outp = att_ps.tile([128, SB], f32, name="outT_ps")
                lp = misc_ps.tile([1, SB], f32, name="l_ps", tag="misc")
                npair = nk // 2

                def qk_pair(t):
                    sp = pj_ps.tile([128, 2, SB], f32, name="s_psum", tag="pj")
                    for u in range(2):
                        nc.tensor.matmul(sp[:, u, :],
                                         kT_all[:, (2 * t + u) * 128:
                                                (2 * t + u + 1) * 128],
                                         q_rhs, start=True, stop=True)
                    return sp

                sp = qk_pair(0)
                for t in range(npair):
                    pt = p_pool.tile([128, 2, SB], bf16, name="p_t")
                    nc.scalar.activation(pt[:], sp[:], Exp, scale=SCALE)
                    for u in range(2):
                        i = 2 * t + u
                        if i >= 4 * j:
                            nc.vector.tensor_tensor(
                                out=pt[:, u, :], in0=pt[:, u, :],
                                in1=masks[:, i - 4 * j, :], op=mult)
                    if t + 1 < npair:
                        sp = qk_pair(t + 1)
                    for u in range(2):
                        i = 2 * t + u
                        first, last = i == 0, i == nk - 1
                        nc.tensor.matmul(outp[:],
                                         v_all[:, i * 128:(i + 1) * 128],
                                         pt[:, u, :], start=first, stop=last)
                    for u in range(2):
                        i = 2 * t + u
                        first, last = i == 0, i == nk - 1
                        nc.tensor.matmul(lp[:], ones_col[:], pt[:, u, :],
                                         start=first, stop=last)

                rcf = tmp.tile([1, SB], f32, name="recipf")
                nc.vector.reciprocal_approx_fast(out=rcf[:], in_=lp[:])
                rc = tmp.tile([1, SB], bf16, name="recip")
                nc.vector.tensor_copy(rc[:], rcf[:])
                pending.append((h, outp, rc))
                if len(pending) > 1:
                    stage_b(*pending.pop(0))
            while pending:
                stage_b(*pending.pop(0))

        emit_oproj(NSB - 1)


def _build():
    if "nc" in _cache:
        return _cache["nc"]
    import concourse.bacc as bacc
    import concourse.bass as bass
    import concourse.tile as tile
    from concourse import mybir

    nc = bacc.Bacc("TRN2", target_bir_lowering=False, debug=False,
                   enable_asserts=False, num_devices=NCORES)
    with tile.TileContext(nc) as tc:
        _emit_program(nc, tc, mybir, bass)
    nc.compile()
    _cache["nc"] = nc
    return nc


def _host_consts():
    rperm = np.zeros((128, 128), np.float32)
    half = D // 2
    for d in range(half):
        rperm[d + half, d] = -1.0      # out[d] = -q[d+64]  (d < 64)
        rperm[d, d + half] = 1.0       # out[d+64] = q[d]
    ident = np.eye(128, dtype=BF16)
    ones_col = np.ones((128, 1), BF16)
    ones_row = np.ones((1, 128), BF16)
    nmr = SB // 128
    masks = np.zeros((nmr, 128, SB), np.float32)
    kk = np.arange(128)[:, None]
    qq = np.arange(SB)[None, :]
    for r in range(nmr):
        masks[r] = (128 * r + kk <= qq).astype(np.float32)
    return {
        "rperm": rperm,
        "ident": ident,
        "ones_col": ones_col,
        "ones_row": ones_row,
        "masks": masks.astype(BF16),
    }


def make_in_maps(hidden_states, cos, sin, Wq, Wk, Wv, Wo):
    hs = np.asarray(hidden_states, np.float32).reshape(S, H)
    hsT = np.ascontiguousarray(hs.T).astype(BF16)
    cosT = np.ascontiguousarray(np.asarray(cos, np.float32).reshape(S, D).T).astype(BF16)
    sinT = np.ascontiguousarray(np.asarray(sin, np.float32).reshape(S, D).T).astype(BF16)
    Wq = np.asarray(Wq, np.float32)
    Wk = np.asarray(Wk, np.float32)
    Wv = np.asarray(Wv, np.float32)
    Wo = np.asarray(Wo, np.float32)
    consts = _host_consts()
    in_maps = []
    for c in range(NCORES):
        m = dict(consts)
        m["hsT"] = hsT
        m["cosT"] = cosT
        m["sinT"] = sinT
        m["wqT"] = np.ascontiguousarray(Wq[c * DC:(c + 1) * DC].T).astype(BF16)
        m["wkT"] = np.ascontiguousarray(Wk[c * D:(c + 1) * D].T).astype(BF16)
        m["wvT"] = np.ascontiguousarray(Wv[c * D:(c + 1) * D].T).astype(BF16)
        m["woT"] = np.ascontiguousarray(Wo[:, c * DC:(c + 1) * DC].T).astype(BF16)
        in_maps.append(m)
    return in_maps


def run_spmd(in_maps, trace=False):
    nc = _build()
    from concourse import bass_utils
    if trace:
        _install_profile_shim()
    return bass_utils.run_bass_kernel_spmd(
        nc, in_maps, core_ids=list(range(NCORES)), trace=trace)


def _install_profile_shim():
    """antenv.axon_hooks is missing from this image; inject an equivalent so
    run_bass_kernel_spmd(trace=True) can reach the libaxon NTFF profiler."""
    import types
    if "antenv.axon_hooks" in sys.modules:
        return
    mod = types.ModuleType("antenv.axon_hooks")
    state = {"hook": None}
    mod.set_axon_ntff_profile_hook = lambda h: state.__setitem__("hook", h)
    mod.get_axon_ntff_profile_hook = lambda: state["hook"]
    sys.modules["antenv.axon_hooks"] = mod
    try:
        import antenv
        antenv.axon_hooks = mod
        if "/root/.axon_site" not in sys.path:
            sys.path.insert(0, "/root/.axon_site")
        from trn_agent_boot.trn_boot import _ntff_profile_via_ctypes
        hook = _ntff_profile_via_ctypes("/opt/axon/libaxon_pjrt.so")
        if hook is not None:
            mod.set_axon_ntff_profile_hook(hook)
    except Exception:
        pass


def kernel(hidden_states, cos, sin, Wq, Wk, Wv, Wo):
    in_maps = make_in_maps(hidden_states, cos, sin, Wq, Wk, Wv, Wo)
    res = run_spmd(in_maps)
    total = np.zeros((S, H), np.float64)
    for c in range(NCORES):
        total += np.asarray(res.results[c]["out"], np.float64)
    return total.astype(np.float32).reshape(1, S, H)


if __name__ == "__main__":
    rng = np.random.default_rng(0)
    ins = {
        "hidden_states": rng.standard_normal((1, S, H)).astype(np.float32),
        "cos": rng.random((1, S, D)).astype(np.float32),
        "sin": rng.random((1, S, D)).astype(np.float32),
        "Wq": (rng.standard_normal((NQH * D, H)) * 0.02).astype(np.float32),
        "Wk": (rng.standard_normal((NKVH * D, H)) * 0.02).astype(np.float32),
        "Wv": (rng.standard_normal((NKVH * D, H)) * 0.02).astype(np.float32),
        "Wo": (rng.standard_normal((H, NQH * D)) * 0.02).astype(np.float32),
    }
    out = kernel(**ins)
    print("out", out.shape, out.dtype, np.abs(out).mean())
